# revision 56
# baseline (speedup 1.0000x reference)
"""Trainium2 Bass kernel for LocalDenseSynthesizerAttention.

Data-parallel over batch B=8 -> 8 cores, one batch each. All projections in
bf16 (PE full rate), fp32 PSUM accumulation. The local window C=45 weighted
sum is computed as banded matmuls.

Band matrix load: attn rows are written to a DRAM scratch `apad` with a row
pitch of 8*128+1 = 1025 elements. With that pitch, the sheared+transposed
band matrix for one 64-t' block and ALL 8 heads is a single CONTIGUOUS
128KB DRAM region read by one XBAR transpose-DMA (32 total):
  b8[w, 8t'+h] = apad[t0+t', h*128 + (w-t')]
because (t0+t')*1025 + 128*(8t'+h) + w = (t0+t')*1025 + h*128 + (w-t').
Out-of-band slots land in the per-head zero pad [45,128) or the zeroed
pitch column, so the matmul sees exact zeros.

The projected v lives only in SBUF, in TWO partition-shifted layouts
(written straight from PSUM with partition-shifted DVE/ACT copies):
  v_A tile m: partitions <-> v rows [128m-22, 128m+106)   (even band blocks)
  v_B tile m: partitions <-> v rows [128m+42, 128m+170)   (odd band blocks)
so every band block is ONE k=108 matmul per head pair with both operands at
base partition 0. No vpad DRAM round-trip, nothing on the SWDGE queue.

All transposes go on the single sync queue: the XBAR is one shared
resource - concurrent transposes from two HWDGE queues corrupt each other.
Same-queue ordering also guarantees the apad-write -> transpose-read
dependency, which Tile does not track for raw DRAM access patterns.

Self-contained: hardcodes shapes from the problem spec.
"""
import sys
sys.path.insert(0, '/opt/trn_rl_repo')
import numpy as np
import ml_dtypes

import concourse.bass as bass
import concourse.mybir as mybir
import concourse.tile as tile
from concourse import bacc
from concourse.bass_utils import run_bass_kernel_spmd

T, F = 2048, 512
H, C, DK = 8, 45, 64
HC = H * C          # 360
W = 128             # per-head band width in apad rows
PITCH = H * W + 1   # 1025: apad row pitch making band reads contiguous
S = 64              # t' band-block size
NB = T // S         # 32 band blocks
PADV = 22           # (C-1)//2
KB = 108            # band matmul contraction: 64 + C - 1
KF = F // 128       # 4 contraction chunks
NT128 = T // 128    # 16

BF16 = mybir.dt.bfloat16
FP8 = mybir.dt.float8e4
F32 = mybir.dt.float32
DR = mybir.MatmulPerfMode.DoubleRow

_CACHE = {}


def _emit_outproj(nc, psm, wk, xhd, wo_t, out, tb):
    ps4 = psm.tile([128, 512], F32, tag="mm", name=f"psE{tb}")
    for p in range(KF):
        nc.tensor.matmul(
            ps4[:], xhd[:, p, tb * 128:(tb + 1) * 128],
            wo_t[:, p, :],
            start=(p == 0), stop=(p == KF - 1))
    o_sb = wk.tile([128, F], BF16, tag="osb", name=f"osb{tb}")
    nc.vector.tensor_copy(out=o_sb[:], in_=ps4[:])
    nc.scalar.dma_start(out[tb * 128:(tb + 1) * 128, :], o_sb[:])


def _build():
    nc = bacc.Bacc("TRN2", target_bir_lowering=False, debug=False, num_devices=8)
    qT = nc.dram_tensor("qT", (F, T), BF16, kind="ExternalInput")
    vT = nc.dram_tensor("vT", (F, T), BF16, kind="ExternalInput")
    w1 = nc.dram_tensor("w1", (F, F), BF16, kind="ExternalInput")
    w2 = nc.dram_tensor("w2", (F, HC), BF16, kind="ExternalInput")
    w3 = nc.dram_tensor("w3", (F, F), BF16, kind="ExternalInput")
    wo = nc.dram_tensor("wo", (F, F), BF16, kind="ExternalInput")
    out = nc.dram_tensor("out", (T, F), BF16, kind="ExternalOutput")

    with tile.TileContext(nc) as tc:
        with tc.tile_pool(name="wpool", bufs=1) as wp, \
             tc.tile_pool(name="inpool", bufs=1) as inp, \
             tc.tile_pool(name="persist", bufs=1) as pers, \
             tc.tile_pool(name="work", bufs=2) as wk, \
             tc.tile_pool(name="band", bufs=10) as bp, \
             tc.tile_pool(name="psmain", bufs=4, space="PSUM") as psm, \
             tc.tile_pool(name="psband", bufs=4, space="PSUM") as psb, \
             tc.tile_pool(name="drampool", bufs=1, space="DRAM") as dp:

            # ---- weights to SBUF, [128, KF, n] layout (partition = contraction)
            w1_t = wp.tile([128, KF, F], BF16, tag="w1")
            nc.scalar.dma_start(w1_t[:], w1[:, :].rearrange("(ko p) n -> p ko n", p=128))
            w2_t = wp.tile([128, KF, HC], BF16, tag="w2")
            nc.scalar.dma_start(w2_t[:], w2[:, :].rearrange("(ko p) n -> p ko n", p=128))
            w3_t = wp.tile([128, KF, F], BF16, tag="w3")
            nc.scalar.dma_start(w3_t[:], w3[:, :].rearrange("(ko p) n -> p ko n", p=128))
            wo_t = wp.tile([128, KF, F], BF16, tag="wo")
            nc.scalar.dma_start(wo_t[:], wo[:, :].rearrange("(ko p) n -> p ko n", p=128))

            # ---- inputs (f-major) to SBUF: 4 folds of [128, T] each
            qT_t = inp.tile([128, KF, T], BF16, tag="qT")
            nc.scalar.dma_start(qT_t[:], qT[:, :].rearrange("(ko p) n -> p ko n", p=128))
            vT_t = inp.tile([128, KF, T], BF16, tag="vT")
            nc.scalar.dma_start(vT_t[:], vT[:, :].rearrange("(ko p) n -> p ko n", p=128))

            # ---- DRAM scratch: attn rows at pitch 1025
            apad = dp.tile([T, PITCH], BF16)
            # v scratch with 22 top + 42 bottom zero guard rows
            vtmp = dp.tile([PADV + T + 42, F], BF16)

            # ---- persistent SBUF activations
            qrT = pers.tile([128, KF, T], BF16, tag="qrT")   # relu(q @ w1), f-major
            # x in head-major layout: chunk p holds hd rows [128p, 128p+128)
            xhd = pers.tile([128, KF, T], BF16, tag="xhd")
            # two partition-shifted v layouts, loaded from vtmp DRAM below
            # (engines can't do unaligned partition bases; DRAM reads can)
            v_A = pers.tile([128, NT128, F], BF16, tag="vA")
            v_B = pers.tile([128, NT128, F], BF16, tag="vB")
            z_t = pers.tile([64, F], BF16, tag="zt")
            nc.vector.memzero(z_t[:])
            nc.gpsimd.dma_start(vtmp[0:PADV, :], z_t[0:PADV, :])
            nc.gpsimd.dma_start(vtmp[PADV + T:PADV + T + 42, :], z_t[0:42, :])

            # ===== Phases A+C interleaved (both PE-dense, independent) ========
            def emit_a(i):
                fo, tt = i // KF, i % KF
                ps = psm.tile([128, 512], F32, tag="mm", name=f"psA{i}")
                for k in range(KF):
                    nc.tensor.matmul(
                        ps[:], w1_t[:, k, fo * 128:(fo + 1) * 128],
                        qT_t[:, k, tt * 512:(tt + 1) * 512],
                        start=(k == 0), stop=(k == KF - 1))
                nc.scalar.activation(qrT[:, fo, tt * 512:(tt + 1) * 512], ps[:],
                                     mybir.ActivationFunctionType.Relu)

            def emit_c(tb):
                ps = psm.tile([128, 512], F32, tag="mm", name=f"psC{tb}")
                for k in range(KF):
                    nc.tensor.matmul(
                        ps[:], vT_t[:, k, tb * 128:(tb + 1) * 128],
                        w3_t[:, k, :],
                        start=(k == 0), stop=(k == KF - 1))
                v_sb = wk.tile([128, F], BF16, tag="vsb", name=f"vsb{tb}")
                nc.vector.tensor_copy(out=v_sb[:], in_=ps[:])
                nc.gpsimd.dma_start(vtmp[PADV + tb * 128:PADV + (tb + 1) * 128, :],
                                    v_sb[:])

            # C lags 8 tiles so its vT load (after qT) is done when C(0) runs
            for i in range(16):
                emit_a(i)
                if i >= 8:
                    emit_c(i - 8)
            for tb in range(8, NT128):
                emit_c(tb)

            # shifted layouts: two big strided reads (overlap phase B).
            # v_A[p, m] = v[128m - 22 + p] = vtmp[128m + p]
            # v_B[p, m] = v[128m + 42 + p] = vtmp[128m + 64 + p]
            vtmp_h = vtmp.tensor
            vtmp_off = vtmp.offset if isinstance(vtmp.offset, int) else 0
            src_A = bass.AP(tensor=vtmp_h, offset=vtmp_off,
                            ap=[[F, 128], [128 * F, NT128], [1, F]])
            nc.gpsimd.dma_start(v_A[:], src_A)
            src_B = bass.AP(tensor=vtmp_h, offset=vtmp_off + 64 * F,
                            ap=[[F, 128], [128 * F, NT128], [1, F]])
            nc.gpsimd.dma_start(v_B[:], src_B)

            # ====== Phases B+D+E interleaved: per 128-row t-block emit the
            # softmax block, the now-ready band transposes, and (lagged 2)
            # the band-matmul unit - PE never drains between phases ========
            apad_h = apad.tensor
            apad_off = apad.offset if isinstance(apad.offset, int) else 0
            b8s = [None] * NB

            def emit_unit(u):
                # one PSUM bank holds two pairs: pair p at cols (p%2)*256
                pss2 = [psb.tile([128, 512], F32, tag="px", name=f"px{u}_{qi}")
                        for qi in range(2)]
                for jj in range(2):
                    bi = 2 * u + jj
                    vsh = v_A if jj == 0 else v_B
                    b8 = b8s[bi]
                    for p in range(4):      # head pairs
                        for i in range(2):
                            h = 2 * p + i
                            # lhsT = vshift head-pair [108, 128]; valid out
                            # rows are [i*64:(i+1)*64]; other half is garbage,
                            # ignored at copyback.
                            c0 = (p % 2) * 256 + jj * 128 + i * 64
                            nc.tensor.matmul(
                                pss2[p // 2][:, c0:c0 + 64],
                                vsh[0:KB, u, p * 128:(p + 1) * 128],
                                b8[0:KB].rearrange("w (t h) -> w h t", h=H)[:, h, :],
                                start=True, stop=True)
                # copy valid halves -> xhd (head-major): bank q covers chunks
                # 2q,2q+1; rows 0:63 take i=0 cols, rows 64:127 take i=1 cols
                for q in range(2):
                    ps4d = pss2[q][:] \
                        .rearrange("d (P j i k) -> d P j i k", P=2, j=2, i=2)
                    dst = xhd[:, 2 * q:2 * q + 2, u * 128:(u + 1) * 128] \
                        .rearrange("d c (j k) -> d c j k", j=2)
                    # split across DVE and ACT so neither queue paces the unit
                    nc.vector.tensor_copy(out=dst[0:64], in_=ps4d[0:64, :, :, 0, :])
                    nc.scalar.copy(dst[64:128], ps4d[64:128, :, :, 1, :])
                # out-proj, lagged 2 units so its PE/queue slots never wait
                # on freshly produced copies
                if u >= 2:
                    _emit_outproj(nc, psm, wk, xhd, wo_t, out, u - 2)

            for tb in range(NT128):
                ps2 = psm.tile([128, 512], F32, tag="mm")
                for k in range(KF):
                    nc.tensor.matmul(
                        ps2[:, 0:HC], qrT[:, k, tb * 128:(tb + 1) * 128],
                        w2_t[:, k, :],
                        start=(k == 0), stop=(k == KF - 1))
                e_t = wk.tile([128, HC], F32, tag="et")
                nc.scalar.activation(e_t[:], ps2[:, 0:HC],
                                     mybir.ActivationFunctionType.Exp)
                zs = wk.tile([128, H], F32, tag="zs")
                nc.vector.reduce_sum(zs[:], e_t[:].rearrange("p (h c) -> p h c", c=C),
                                     axis=mybir.AxisListType.X)
                rz = wk.tile([128, H], F32, tag="rz")
                nc.vector.reciprocal(rz[:], zs[:])
                ap_t = wk.tile([128, PITCH + 1], BF16, tag="apad")
                if tb < 2:
                    # zero the pad region once per pool slot (bufs=2); only
                    # cols [h*128, h*128+45) are overwritten afterwards
                    nc.vector.memzero(ap_t[:])
                nc.vector.tensor_mul(
                    out=ap_t[:, 0:H * W].rearrange("p (h w) -> p h w", w=W)[:, :, 0:C],
                    in0=e_t[:].rearrange("p (h c) -> p h c", c=C),
                    in1=rz[:, :, None].to_broadcast((128, H, C)))
                # same queue as the band transposes that read apad: queue
                # order backs up the dependency tracking
                nc.sync.dma_start(apad[tb * 128:(tb + 1) * 128, :], ap_t[:, 0:PITCH])

                # launch the band transposes whose apad rows just landed
                # (t(bi) needs apad blocks up to (bi+1)//2)
                bis = [0] if tb == 0 else [2 * tb - 1, 2 * tb]
                if tb == NT128 - 1:
                    bis.append(NB - 1)
                for bi in bis:
                    b8 = bp.tile([128, H * S], BF16, tag="b8", name=f"b8_{bi}")
                    b8s[bi] = b8
                    srcap = bass.AP(
                        tensor=apad_h,
                        offset=apad_off + S * bi * PITCH,
                        ap=[[W, H * S], [1, W]])
                    nc.sync.dma_start_transpose(b8[:], srcap)

                # band unit lagged 2 blocks behind the softmax
                if tb >= 2:
                    emit_unit(tb - 2)

            emit_unit(NT128 - 2)
            emit_unit(NT128 - 1)
            _emit_outproj(nc, psm, wk, xhd, wo_t, out, NT128 - 2)
            _emit_outproj(nc, psm, wk, xhd, wo_t, out, NT128 - 1)

    nc.compile()
    return nc


def _get_nc():
    if "nc" not in _CACHE:
        _CACHE["nc"] = _build()
    return _CACHE["nc"]


def kernel(query, key, value, w1, w2, w3, w_out, _trace=False):
    query = np.asarray(query)
    value = np.asarray(value)
    nc = _get_nc()
    bf = ml_dtypes.bfloat16
    w1b = np.ascontiguousarray(np.asarray(w1)).astype(bf)
    w2b = np.ascontiguousarray(np.asarray(w2)).astype(bf)
    w3b = np.ascontiguousarray(np.asarray(w3)).astype(bf)
    wob = np.ascontiguousarray(np.asarray(w_out)).astype(bf)
    in_maps = []
    for b in range(8):
        in_maps.append({
            "qT": np.ascontiguousarray(query[b].T).astype(bf),
            "vT": np.ascontiguousarray(value[b].T).astype(bf),
            "w1": w1b, "w2": w2b, "w3": w3b, "wo": wob,
        })
    res = run_bass_kernel_spmd(nc, in_maps, list(range(8)), trace=_trace)
    if _trace:
        _CACHE["last_result"] = res
    out = np.stack([res.results[b]["out"] for b in range(8)], axis=0)
    return out.astype(np.float32)


# revision 57
# speedup vs baseline: 1.0988x; 1.0988x over previous
"""Trainium2 Bass kernel for LocalDenseSynthesizerAttention.

Data-parallel over batch B=8 -> 8 cores, one batch each. All projections in
bf16 (PE full rate), fp32 PSUM accumulation. The local window C=45 weighted
sum is computed as banded matmuls.

Band matrix load: attn rows are written to a DRAM scratch `apad` with a row
pitch of 8*128+1 = 1025 elements. With that pitch, the sheared+transposed
band matrix for one 64-t' block and ALL 8 heads is a single CONTIGUOUS
128KB DRAM region read by one XBAR transpose-DMA (32 total):
  b8[w, 8t'+h] = apad[t0+t', h*128 + (w-t')]
because (t0+t')*1025 + 128*(8t'+h) + w = (t0+t')*1025 + h*128 + (w-t').
Out-of-band slots land in the per-head zero pad [45,128) or the zeroed
pitch column, so the matmul sees exact zeros.

The projected v lives only in SBUF, in TWO partition-shifted layouts
(written straight from PSUM with partition-shifted DVE/ACT copies):
  v_A tile m: partitions <-> v rows [128m-22, 128m+106)   (even band blocks)
  v_B tile m: partitions <-> v rows [128m+42, 128m+170)   (odd band blocks)
so every band block is ONE k=108 matmul per head pair with both operands at
base partition 0. No vpad DRAM round-trip, nothing on the SWDGE queue.

All transposes go on the single sync queue: the XBAR is one shared
resource - concurrent transposes from two HWDGE queues corrupt each other.
Same-queue ordering also guarantees the apad-write -> transpose-read
dependency, which Tile does not track for raw DRAM access patterns.

Self-contained: hardcodes shapes from the problem spec.
"""
import sys
sys.path.insert(0, '/opt/trn_rl_repo')
import numpy as np
import ml_dtypes

import concourse.bass as bass
import concourse.mybir as mybir
import concourse.tile as tile
from concourse import bacc
from concourse.bass_utils import run_bass_kernel_spmd

T, F = 2048, 512
H, C, DK = 8, 45, 64
HC = H * C          # 360
W = 128             # per-head band width in apad rows
PITCH = H * W + 1   # 1025: apad row pitch making band reads contiguous
S = 64              # t' band-block size
NB = T // S         # 32 band blocks
PADV = 22           # (C-1)//2
KB = 108            # band matmul contraction: 64 + C - 1
KF = F // 128       # 4 contraction chunks
NT128 = T // 128    # 16

BF16 = mybir.dt.bfloat16
FP8 = mybir.dt.float8e4
F32 = mybir.dt.float32
DR = mybir.MatmulPerfMode.DoubleRow

_CACHE = {}


def _emit_outproj(nc, psm, wk, xhd, wo_t, out, tb):
    ps4 = psm.tile([128, 512], F32, tag="mm", name=f"psE{tb}")
    for p in range(KF):
        nc.tensor.matmul(
            ps4[:], xhd[:, p, tb * 128:(tb + 1) * 128],
            wo_t[:, p, :],
            start=(p == 0), stop=(p == KF - 1))
    o_sb = wk.tile([128, F], BF16, tag="osb", name=f"osb{tb}")
    nc.vector.tensor_copy(out=o_sb[:], in_=ps4[:])
    nc.scalar.dma_start(out[tb * 128:(tb + 1) * 128, :], o_sb[:])


def _build():
    nc = bacc.Bacc("TRN2", target_bir_lowering=False, debug=False, num_devices=8)
    qT = nc.dram_tensor("qT", (F, T), BF16, kind="ExternalInput")
    vT = nc.dram_tensor("vT", (F, T), BF16, kind="ExternalInput")
    w1 = nc.dram_tensor("w1", (F, F), BF16, kind="ExternalInput")
    w2 = nc.dram_tensor("w2", (F, HC), BF16, kind="ExternalInput")
    w3 = nc.dram_tensor("w3", (F, F), BF16, kind="ExternalInput")
    wo = nc.dram_tensor("wo", (F, F), BF16, kind="ExternalInput")
    out = nc.dram_tensor("out", (T, F), BF16, kind="ExternalOutput")

    with tile.TileContext(nc) as tc:
        with tc.tile_pool(name="wpool", bufs=1) as wp, \
             tc.tile_pool(name="inpool", bufs=1) as inp, \
             tc.tile_pool(name="persist", bufs=1) as pers, \
             tc.tile_pool(name="work", bufs=2) as wk, \
             tc.tile_pool(name="band", bufs=10) as bp, \
             tc.tile_pool(name="psmain", bufs=4, space="PSUM") as psm, \
             tc.tile_pool(name="psband", bufs=4, space="PSUM") as psb, \
             tc.tile_pool(name="drampool", bufs=1, space="DRAM") as dp:

            # ---- weights to SBUF, [128, KF, n] layout (partition = contraction)
            w1_t = wp.tile([128, KF, F], BF16, tag="w1")
            nc.scalar.dma_start(w1_t[:], w1[:, :].rearrange("(ko p) n -> p ko n", p=128))
            w2_t = wp.tile([128, KF, HC], BF16, tag="w2")
            nc.scalar.dma_start(w2_t[:], w2[:, :].rearrange("(ko p) n -> p ko n", p=128))
            w3_t = wp.tile([128, KF, F], BF16, tag="w3")
            nc.scalar.dma_start(w3_t[:], w3[:, :].rearrange("(ko p) n -> p ko n", p=128))
            wo_t = wp.tile([128, KF, F], BF16, tag="wo")
            nc.scalar.dma_start(wo_t[:], wo[:, :].rearrange("(ko p) n -> p ko n", p=128))

            # ---- inputs (f-major) to SBUF: 4 folds of [128, T] each
            qT_t = inp.tile([128, KF, T], BF16, tag="qT")
            nc.scalar.dma_start(qT_t[:], qT[:, :].rearrange("(ko p) n -> p ko n", p=128))
            vT_t = inp.tile([128, KF, T], BF16, tag="vT")
            nc.scalar.dma_start(vT_t[:], vT[:, :].rearrange("(ko p) n -> p ko n", p=128))

            # ---- DRAM scratch: attn rows at pitch 1025
            apad = dp.tile([T, PITCH], BF16)
            # v scratch with 22 top + 42 bottom zero guard rows
            vtmp = dp.tile([PADV + T + 42, F], BF16)

            # ---- persistent SBUF activations
            qrT = pers.tile([128, KF, T], BF16, tag="qrT")   # relu(q @ w1), f-major
            # x in head-major layout: chunk p holds hd rows [128p, 128p+128)
            xhd = pers.tile([128, KF, T], BF16, tag="xhd")
            # two partition-shifted v layouts, loaded from vtmp DRAM below
            # (engines can't do unaligned partition bases; DRAM reads can)
            v_A = pers.tile([128, NT128, F], BF16, tag="vA")
            v_B = pers.tile([128, NT128, F], BF16, tag="vB")
            z_t = pers.tile([64, F], BF16, tag="zt")
            nc.vector.memzero(z_t[:])
            nc.gpsimd.dma_start(vtmp[0:PADV, :], z_t[0:PADV, :])
            nc.gpsimd.dma_start(vtmp[PADV + T:PADV + T + 42, :], z_t[0:42, :])

            # ===== Phases A+C interleaved (both PE-dense, independent) ========
            def emit_a(i):
                fo, tt = i // KF, i % KF
                ps = psm.tile([128, 512], F32, tag="mm", name=f"psA{i}")
                for k in range(KF):
                    nc.tensor.matmul(
                        ps[:], w1_t[:, k, fo * 128:(fo + 1) * 128],
                        qT_t[:, k, tt * 512:(tt + 1) * 512],
                        start=(k == 0), stop=(k == KF - 1))
                nc.scalar.activation(qrT[:, fo, tt * 512:(tt + 1) * 512], ps[:],
                                     mybir.ActivationFunctionType.Relu)

            def emit_c(tb):
                ps = psm.tile([128, 512], F32, tag="mm", name=f"psC{tb}")
                for k in range(KF):
                    nc.tensor.matmul(
                        ps[:], vT_t[:, k, tb * 128:(tb + 1) * 128],
                        w3_t[:, k, :],
                        start=(k == 0), stop=(k == KF - 1))
                v_sb = wk.tile([128, F], BF16, tag="vsb", name=f"vsb{tb}")
                nc.vector.tensor_copy(out=v_sb[:], in_=ps[:])
                nc.gpsimd.dma_start(vtmp[PADV + tb * 128:PADV + (tb + 1) * 128, :],
                                    v_sb[:])

            # shifted layouts: strided reads in tile-halves so early band
            # units don't wait for the whole of phase C.
            # v_A[p, m] = v[128m - 22 + p] = vtmp[128m + p]
            # v_B[p, m] = v[128m + 42 + p] = vtmp[128m + 64 + p]
            vtmp_h = vtmp.tensor
            vtmp_off = vtmp.offset if isinstance(vtmp.offset, int) else 0

            def emit_vshift(m0, m1):
                n = m1 - m0
                src_A = bass.AP(tensor=vtmp_h, offset=vtmp_off + m0 * 128 * F,
                                ap=[[F, 128], [128 * F, n], [1, F]])
                nc.gpsimd.dma_start(v_A[:, m0:m1, :], src_A)
                src_B = bass.AP(tensor=vtmp_h,
                                offset=vtmp_off + m0 * 128 * F + 64 * F,
                                ap=[[F, 128], [128 * F, n], [1, F]])
                nc.gpsimd.dma_start(v_B[:, m0:m1, :], src_B)

            # C lags 8 tiles so its vT load (after qT) is done when C(0) runs
            for i in range(16):
                emit_a(i)
                if i >= 8:
                    emit_c(i - 8)
            for tb in range(8, NT128):
                emit_c(tb)
                if tb == 8:
                    # v tiles 0..6 only need vtmp rows < 128*7+106 -> writes
                    # tb<=7 done (plus guards); gpsimd queue order covers it
                    emit_vshift(0, 7)
            emit_vshift(7, NT128)

            # ====== Phases B+D+E interleaved: per 128-row t-block emit the
            # softmax block, the now-ready band transposes, and (lagged 2)
            # the band-matmul unit - PE never drains between phases ========
            apad_h = apad.tensor
            apad_off = apad.offset if isinstance(apad.offset, int) else 0
            b8s = [None] * NB

            def emit_unit(u):
                # one PSUM bank holds two pairs: pair p at cols (p%2)*256
                pss2 = [psb.tile([128, 512], F32, tag="px", name=f"px{u}_{qi}")
                        for qi in range(2)]
                for jj in range(2):
                    bi = 2 * u + jj
                    vsh = v_A if jj == 0 else v_B
                    b8 = b8s[bi]
                    for p in range(4):      # head pairs
                        for i in range(2):
                            h = 2 * p + i
                            # lhsT = vshift head-pair [108, 128]; valid out
                            # rows are [i*64:(i+1)*64]; other half is garbage,
                            # ignored at copyback.
                            c0 = (p % 2) * 256 + jj * 128 + i * 64
                            nc.tensor.matmul(
                                pss2[p // 2][:, c0:c0 + 64],
                                vsh[0:KB, u, p * 128:(p + 1) * 128],
                                b8[0:KB].rearrange("w (t h) -> w h t", h=H)[:, h, :],
                                start=True, stop=True)
                # copy valid halves -> xhd (head-major): bank q covers chunks
                # 2q,2q+1; rows 0:63 take i=0 cols, rows 64:127 take i=1 cols
                for q in range(2):
                    ps4d = pss2[q][:] \
                        .rearrange("d (P j i k) -> d P j i k", P=2, j=2, i=2)
                    dst = xhd[:, 2 * q:2 * q + 2, u * 128:(u + 1) * 128] \
                        .rearrange("d c (j k) -> d c j k", j=2)
                    # split across DVE and ACT so neither queue paces the unit
                    nc.vector.tensor_copy(out=dst[0:64], in_=ps4d[0:64, :, :, 0, :])
                    nc.scalar.copy(dst[64:128], ps4d[64:128, :, :, 1, :])
                # out-proj, lagged 2 units so its PE/queue slots never wait
                # on freshly produced copies
                if u >= 2:
                    _emit_outproj(nc, psm, wk, xhd, wo_t, out, u - 2)

            for tb in range(NT128):
                ps2 = psm.tile([128, 512], F32, tag="mm")
                for k in range(KF):
                    nc.tensor.matmul(
                        ps2[:, 0:HC], qrT[:, k, tb * 128:(tb + 1) * 128],
                        w2_t[:, k, :],
                        start=(k == 0), stop=(k == KF - 1))
                e_t = wk.tile([128, HC], F32, tag="et")
                nc.scalar.activation(e_t[:], ps2[:, 0:HC],
                                     mybir.ActivationFunctionType.Exp)
                zs = wk.tile([128, H], F32, tag="zs")
                nc.vector.reduce_sum(zs[:], e_t[:].rearrange("p (h c) -> p h c", c=C),
                                     axis=mybir.AxisListType.X)
                rz = wk.tile([128, H], F32, tag="rz")
                nc.vector.reciprocal(rz[:], zs[:])
                ap_t = wk.tile([128, PITCH + 1], BF16, tag="apad")
                if tb < 2:
                    # zero the pad region once per pool slot (bufs=2); only
                    # cols [h*128, h*128+45) are overwritten afterwards
                    nc.vector.memzero(ap_t[:])
                nc.vector.tensor_mul(
                    out=ap_t[:, 0:H * W].rearrange("p (h w) -> p h w", w=W)[:, :, 0:C],
                    in0=e_t[:].rearrange("p (h c) -> p h c", c=C),
                    in1=rz[:, :, None].to_broadcast((128, H, C)))
                # same queue as the band transposes that read apad: queue
                # order backs up the dependency tracking
                nc.sync.dma_start(apad[tb * 128:(tb + 1) * 128, :], ap_t[:, 0:PITCH])

                # launch the band transposes whose apad rows just landed
                # (t(bi) needs apad blocks up to (bi+1)//2)
                bis = [0] if tb == 0 else [2 * tb - 1, 2 * tb]
                if tb == NT128 - 1:
                    bis.append(NB - 1)
                for bi in bis:
                    b8 = bp.tile([128, H * S], BF16, tag="b8", name=f"b8_{bi}")
                    b8s[bi] = b8
                    srcap = bass.AP(
                        tensor=apad_h,
                        offset=apad_off + S * bi * PITCH,
                        ap=[[W, H * S], [1, W]])
                    nc.sync.dma_start_transpose(b8[:], srcap)

                # band unit lagged 2 blocks behind the softmax
                if tb >= 2:
                    emit_unit(tb - 2)

            emit_unit(NT128 - 2)
            emit_unit(NT128 - 1)
            _emit_outproj(nc, psm, wk, xhd, wo_t, out, NT128 - 2)
            _emit_outproj(nc, psm, wk, xhd, wo_t, out, NT128 - 1)

    nc.compile()
    return nc


def _get_nc():
    if "nc" not in _CACHE:
        _CACHE["nc"] = _build()
    return _CACHE["nc"]


def kernel(query, key, value, w1, w2, w3, w_out, _trace=False):
    query = np.asarray(query)
    value = np.asarray(value)
    nc = _get_nc()
    bf = ml_dtypes.bfloat16
    w1b = np.ascontiguousarray(np.asarray(w1)).astype(bf)
    w2b = np.ascontiguousarray(np.asarray(w2)).astype(bf)
    w3b = np.ascontiguousarray(np.asarray(w3)).astype(bf)
    wob = np.ascontiguousarray(np.asarray(w_out)).astype(bf)
    in_maps = []
    for b in range(8):
        in_maps.append({
            "qT": np.ascontiguousarray(query[b].T).astype(bf),
            "vT": np.ascontiguousarray(value[b].T).astype(bf),
            "w1": w1b, "w2": w2b, "w3": w3b, "wo": wob,
        })
    res = run_bass_kernel_spmd(nc, in_maps, list(range(8)), trace=_trace)
    if _trace:
        _CACHE["last_result"] = res
    out = np.stack([res.results[b]["out"] for b in range(8)], axis=0)
    return out.astype(np.float32)
